# revision 5
# baseline (speedup 1.0000x reference)
"""BiLSTM-CRF Trainium2 Bass kernel — sequence-parallel v2.

v2 over v1: each core runs FOUR LSTM streams (2 forward + 2 backward
chunks of 80 valid steps + 40 warmup = 120 steps) packed as two
same-direction PAIRS that share instructions ([128, 2, 64] elementwise,
[128, 2x64] matmul rhs) — halving per-instruction overhead and the
serial chain count. The input projection is fused into the gate PSUM
(two accumulating matmuls per gate) instead of a separate xg phase.

  core q owns steps [128q, 128q+128); em region [128q-32, 128q+128):
    f1: [cs-72, cs+48)   f2: [cs+8, cs+128)    (40-step warmups)
    b1: [cs+88) down to [cs-32)   b2: [cs+168) down to [cs+48)
  Viterbi (unchanged from v1): two 96-step windows on partition halves,
  int32 max-plus with argmax packed in low 5 bits, hist decoded on host.
"""

import numpy as np

import concourse.bass as bass
import concourse.mybir as mybir
from concourse.tile import TileContext
from concourse.bass_utils import run_bass_kernel_spmd

F32 = mybir.dt.float32
F16 = mybir.dt.float16
I32 = mybir.dt.int32
AF = mybir.ActivationFunctionType
ALU = mybir.AluOpType

V, E, H, T = 32000, 100, 128, 17
B, S = 64, 1024
NC = 8
CL = 128                  # chunk per core
WU = 40                   # LSTM warmup steps
VS = [54, 54, 52]         # valid steps per split
V0 = [-32, 22, 76]        # valid-region starts (s_rel)
NSTS = [94, 94, 92]       # stream lengths per pair
NCOL = 240                # token cols: s_rel in [-72, 168)
WOFF = 72                 # col 0 = cs - 72
VW = 96                   # em region assembly span (unused by viterbi)
VW2 = 64                  # viterbi window (32 warmup + 32 valid)
QS = float(1 << 16)
PACK = 32
RNULL = -(1 << 25)
DC = 0.35
PIN = -10000.0


def _split_multi_waits(nc):
    ctr = [0]
    for fn in nc.m.functions:
        for bb in fn.blocks:
            out = []
            changed = False
            for inst in bb.instructions:
                si = inst.sync_info
                waits = list(si.on_wait) if si is not None and si.on_wait else []
                if len(waits) > 1:
                    si.on_wait = waits[:1]
                    for w in waits[1:]:
                        ctr[0] += 1
                        out.append(mybir.InstNoOp(
                            name=f"I-waitfix-{ctr[0]}", ins=[], outs=[],
                            engine=inst.engine,
                            sync_info=mybir.SyncInfo(on_wait=[w], on_update=[]),
                        ))
                    changed = True
                out.append(inst)
            if changed:
                bb.instructions = out


def _build(dbg=False):
    nc = bass.Bass()

    xT_d = nc.dram_tensor("xT", [128, NCOL * B], F16, kind="ExternalInput")
    wih_d = nc.dram_tensor("wih", [2, 4, 128, H], F16, kind="ExternalInput")
    whh_d = nc.dram_tensor("whh", [2, 4, H, H], F16, kind="ExternalInput")
    fcw_d = nc.dram_tensor("fcw", [2, H, T], F16, kind="ExternalInput")
    fcwl_d = nc.dram_tensor("fcwl", [2, H, T], F16, kind="ExternalInput")
    crep_d = nc.dram_tensor("crep", [128, T, T], I32, kind="ExternalInput")
    init_d = nc.dram_tensor("init", [128, T], I32, kind="ExternalInput")
    rstr_d = nc.dram_tensor("rstr", [128, T], I32, kind="ExternalInput")
    rstm_d = nc.dram_tensor("rstm", [128, T], I32, kind="ExternalInput")

    hist_o = nc.dram_tensor("hist_o", [2, 128, VW2 * T], I32,
                            kind="ExternalOutput")
    scf_o = nc.dram_tensor("scf_o", [128, T], I32, kind="ExternalOutput")
    if dbg:
        emdbg_o = nc.dram_tensor("emdbg_o", [128, VW * T], I32,
                                 kind="ExternalOutput")

    with TileContext(nc) as tc:
        import contextlib
        es = contextlib.ExitStack()
        with es:
            cp = es.enter_context(tc.tile_pool(name="consts", bufs=1))

            xT = cp.tile([128, NCOL, B], F16, tag="xT")
            nc.sync.dma_start(out=xT[:, :, :],
                              in_=xT_d[:, :].rearrange("p (c b) -> p c b", b=B))
            wih = cp.tile([128, 2, 4, H], F16, tag="wih")
            whh = cp.tile([128, 2, 4, H], F16, tag="whh")
            for d in range(2):
                for g in range(4):
                    nc.sync.dma_start(out=wih[:, d, g, :], in_=wih_d[d, g, :, :])
                    nc.sync.dma_start(out=whh[:, d, g, :], in_=whh_d[d, g, :, :])
            fcw = cp.tile([H, 2, T], F16, tag="fcw")
            fcwl = cp.tile([H, 2, T], F16, tag="fcwl")
            for d in range(2):
                nc.sync.dma_start(out=fcw[:, d, :], in_=fcw_d[d, :, :])
                nc.sync.dma_start(out=fcwl[:, d, :], in_=fcwl_d[d, :, :])
            crep = cp.tile([128, T, T], I32, tag="crep")
            nc.sync.dma_start(out=crep[:, :, :], in_=crep_d[:, :, :])
            init_t = cp.tile([128, T], I32, tag="init")
            nc.sync.dma_start(out=init_t[:], in_=init_d[:, :])
            rstr = cp.tile([128, T], I32, tag="rstr")
            nc.sync.dma_start(out=rstr[:], in_=rstr_d[:, :])
            rstm = cp.tile([128, T], I32, tag="rstm")
            nc.sync.dma_start(out=rstm[:], in_=rstm_d[:, :])

            # paired h histories: [128, step, stream-in-pair, batch]
            # pair 0 = (f0, f1); pair 1 = (b0, b1); pair 2 = (f2, b2)
            hp = [cp.tile([128, NSTS[p], 2, B], F16, tag=f"hp{p}",
                          name=f"hp{p}") for p in range(3)]
            # hlo only spans post-warmup steps (index = t - WU)
            hlo = [cp.tile([128, NSTS[p] - WU, 2, B], F16, tag=f"hlo{p}",
                           name=f"hlo{p}") for p in range(3)]
            # two interleaved viterbi chains: chain0=(A,C), chain1=(B,D)
            # em col for (chain k, half w, tau) = tau + 32k + 64w
            em32 = [cp.tile([128, VW2, T], I32, tag=f"em32{k}",
                            name=f"em32{k}") for k in range(2)]
            hist = [cp.tile([128, VW2, T], I32, tag=f"hist{k}",
                            name=f"hist{k}") for k in range(2)]

            z2 = cp.tile([128, 2, B], F16, tag="z2")
            nc.vector.memset(z2[:], 0.0)
            cmask = cp.tile([128, 3], I32, tag="cmask")
            nc.vector.memset(cmask[:, 0:1], -PACK)
            nc.vector.memset(cmask[:, 1:2], PACK - 1)
            nc.vector.memset(cmask[:, 2:3], 5)

            c_pp = [[cp.tile([128, 2, B], F32, tag=f"c{d}{i}", name=f"c{d}{i}")
                     for i in range(2)] for d in range(3)]
            for d in range(3):
                nc.vector.memset(c_pp[d][1][:], 0.0)

            # ---- phase 1: paired LSTM streams, x fused into gate PSUM ----
            with tc.tile_pool(name="psg0", bufs=1, space="PSUM") as psg0, \
                 tc.tile_pool(name="psg1", bufs=1, space="PSUM") as psg1, \
                 tc.tile_pool(name="psg2", bufs=1, space="PSUM") as psg2, \
                 tc.tile_pool(name="sm", bufs=3) as sm:
                psg = [psg0, psg1, psg2]

                def emit_step(d, t):
                    # all streams write h at index t; b-streams map to
                    # descending s_rel (handled at read time)
                    hprev = z2[:] if t == 0 else hp[d][:, t - 1, :, :]
                    gps = psg[d].tile([128, 4, 2, B], F32, tag=f"gps{d}")
                    if d == 0:      # (f0, f1): cols {t, 54+t}
                        x_rhs = xT[:, t:t + 55:54, :]
                        for g in range(4):
                            nc.tensor.matmul(gps[:, g, :, :], whh[:, 0, g, :],
                                             hprev, start=True, stop=False)
                            nc.tensor.matmul(gps[:, g, :, :], wih[:, 0, g, :],
                                             x_rhs, start=False, stop=True)
                    elif d == 1:    # (b0, b1): cols {133-t, 187-t}
                        x_rhs = xT[:, 133 - t:188 - t:54, :]
                        for g in range(4):
                            nc.tensor.matmul(gps[:, g, :, :], whh[:, 1, g, :],
                                             hprev, start=True, stop=False)
                            nc.tensor.matmul(gps[:, g, :, :], wih[:, 1, g, :],
                                             x_rhs, start=False, stop=True)
                    else:           # mixed (f2, b2): per-half matmuls
                        hpf = z2[:, 0, :] if t == 0 else hp[2][:, t - 1, 0, :]
                        hpb = z2[:, 1, :] if t == 0 else hp[2][:, t - 1, 1, :]
                        for g in range(4):
                            nc.tensor.matmul(gps[:, g, 0, :], whh[:, 0, g, :],
                                             hpf, start=True, stop=False)
                            nc.tensor.matmul(gps[:, g, 0, :], wih[:, 0, g, :],
                                             xT[:, 108 + t, :],
                                             start=False, stop=True)
                            nc.tensor.matmul(gps[:, g, 1, :], whh[:, 1, g, :],
                                             hpb, start=True, stop=False)
                            nc.tensor.matmul(gps[:, g, 1, :], wih[:, 1, g, :],
                                             xT[:, 239 - t, :],
                                             start=False, stop=True)
                    sig = sm.tile([128, 3, 2, B], F32, tag=f"sig{d}")
                    nc.scalar.activation(sig[:], gps[:, 0:3, :, :], AF.Sigmoid)
                    tg = sm.tile([128, 2, B], F32, tag=f"tg{d}")
                    nc.scalar.activation(tg[:], gps[:, 3, :, :], AF.Tanh)
                    t1 = sm.tile([128, 2, B], F32, tag=f"t1{d}")
                    nc.vector.tensor_mul(t1[:], sig[:, 0, :, :], tg[:])
                    cprev = c_pp[d][(t + 1) % 2]
                    cnew = c_pp[d][t % 2]
                    nc.gpsimd.tensor_mul(cnew[:], sig[:, 1, :, :], cprev[:])
                    nc.vector.tensor_add(cnew[:], cnew[:], t1[:])
                    thc = sm.tile([128, 2, B], F32, tag=f"thc{d}")
                    nc.scalar.activation(thc[:], cnew[:], AF.Tanh)
                    hout = hp[d][:, t, :, :]
                    hlout = hlo[d][:, t - WU, :, :] if t >= WU else None
                    # critical path: h16 written directly by DVE (fp16 out)
                    nc.vector.tensor_mul(hout, sig[:, 2, :, :], thc[:])
                    # off-path fp16 residual for the em correction; em only
                    # reads hlo written at t >= WU, so skip during warmup
                    if t >= WU:
                        h32 = sm.tile([128, 2, B], F32, tag=f"h32{d}")
                        nc.gpsimd.tensor_mul(h32[:], sig[:, 2, :, :], thc[:])
                        nc.vector.tensor_sub(hlout, h32[:], hout)

                for t in range(94):
                    for d in range(3):
                        if t < NSTS[d]:
                            emit_step(d, t)

            # ---- phase 2: emissions (em col c: s_rel = c-32) ----
            # em col c -> (pair, step-index, half)
            def hf_i(c):
                if c < 54:
                    return (0, c + 40, 0)
                if c < 108:
                    return (0, c - 14, 1)
                return (2, c - 68, 0)

            def hb_i(c):
                if c < 54:
                    return (1, 93 - c, 0)
                if c < 108:
                    return (1, 147 - c, 1)
                return (2, 199 - c, 1)

            EMG = 16
            with tc.tile_pool(name="psem", bufs=2, space="PSUM") as psem, \
                 tc.tile_pool(name="emtmp", bufs=2) as emtmp:
                for k in range(2):
                  for g0 in range(0, VW2, EMG):
                    ps = psem.tile([128, EMG, T], F32, tag="psem")
                    for i in range(EMG):
                        tau = g0 + i
                        for w, off in ((0, 0), (1, 64)):
                            c = tau + 32 * k + 64 * w
                            (pf, tf, wf), (pb, tb, wb) = hf_i(c), hb_i(c)
                            pw = ps[off:off + 64, i, :]
                            nc.tensor.matmul(pw, hp[pf][:, tf, wf, :],
                                             fcw[:, 0, :], start=True, stop=False)
                            nc.tensor.matmul(pw, hlo[pf][:, tf - WU, wf, :],
                                             fcw[:, 0, :], start=False, stop=False)
                            nc.tensor.matmul(pw, hp[pf][:, tf, wf, :],
                                             fcwl[:, 0, :], start=False, stop=False)
                            nc.tensor.matmul(pw, hp[pb][:, tb, wb, :],
                                             fcw[:, 1, :], start=False, stop=False)
                            nc.tensor.matmul(pw, hlo[pb][:, tb - WU, wb, :],
                                             fcw[:, 1, :], start=False, stop=False)
                            nc.tensor.matmul(pw, hp[pb][:, tb, wb, :],
                                             fcwl[:, 1, :], start=False, stop=True)
                    tmp = emtmp.tile([128, EMG, T], I32, tag="emtmp")
                    nc.scalar.activation(tmp[:], ps[:], AF.Identity, scale=QS)
                    nc.vector.tensor_scalar(
                        out=em32[k][:, g0:g0 + EMG, :], in0=tmp[:],
                        scalar1=cmask[:, 2:3], scalar2=None,
                        op0=ALU.logical_shift_left)

            if dbg:
                nc.sync.dma_start(
                    out=emdbg_o[:, :],
                    in_=em32[0][:, :, :].rearrange("p a b -> p (a b)"))

            # ---- phase 3: viterbi (identical to v1) ----
            with tc.tile_pool(name="vit", bufs=1) as vp:
                score = [vp.tile([128, T], I32, tag=f"score{k}",
                                 name=f"score{k}") for k in range(2)]
                ns = [vp.tile([128, T, T], I32, tag=f"ns{k}",
                              name=f"ns{k}") for k in range(2)]
                for k in range(2):
                    nc.vector.tensor_tensor(out=score[k][:], in0=init_t[:],
                                            in1=em32[k][:, 0, :], op=ALU.add)
                for tau in range(1, VW2):
                    for k in range(2):
                        nc.gpsimd.tensor_tensor(
                            out=ns[k][:],
                            in0=score[k][:, :].unsqueeze(1)
                                .broadcast_to((128, T, T)),
                            in1=crep[:, :, :], op=ALU.add)
                        p_t = hist[k][:, tau, :]
                        nc.vector.tensor_reduce(out=p_t, in_=ns[k][:],
                                                axis=mybir.AxisListType.X,
                                                op=ALU.max)
                        if tau == 32 and k == 0:
                            # core-0 window A re-init at s=0
                            nc.vector.tensor_tensor(out=score[k][:], in0=p_t,
                                                    in1=rstm[:], op=ALU.add)
                            nc.vector.tensor_tensor(out=score[k][:],
                                                    in0=score[k][:],
                                                    in1=rstr[:], op=ALU.max)
                            p_t = score[k][:, :]
                        nc.vector.tensor_tensor(out=score[k][:], in0=p_t,
                                                in1=em32[k][:, tau, :],
                                                op=ALU.add)
                        nc.vector.tensor_scalar(
                            out=score[k][:], in0=score[k][:],
                            scalar1=cmask[:, 0:1],
                            scalar2=None, op0=ALU.bitwise_and)
                nc.sync.dma_start(out=scf_o[:, :], in_=score[1][:])
                for k in range(2):
                    nc.sync.dma_start(
                        out=hist_o[k, :, :],
                        in_=hist[k][:, :, :].rearrange("p a b -> p (a b)"))

    _split_multi_waits(nc)
    return nc


_NC_CACHE = {}


def _get_nc(dbg=False):
    if dbg not in _NC_CACHE:
        _NC_CACHE[dbg] = _build(dbg)
    return _NC_CACHE[dbg]


def _host_inputs(sentence, embed, w_ih_f, w_hh_f, b_ih_f, b_hh_f,
                 w_ih_b, w_hh_b, b_ih_b, b_hh_b, fc_w, fc_b,
                 start_trans, end_trans, trans):
    f16 = np.float16

    ep = np.zeros((V, 128), np.float32)
    ep[:, :E] = np.asarray(embed, np.float32)
    ep[:, E] = 1.0

    wih = np.zeros((2, 4, 128, H), np.float32)
    whh = np.zeros((2, 4, H, H), np.float32)
    slot2pt = [0, 1, 3, 2]   # slots: i, f, o, g
    for d, (w_ih, w_hh, b_ih, b_hh) in enumerate(
            [(w_ih_f, w_hh_f, b_ih_f, b_hh_f),
             (w_ih_b, w_hh_b, b_ih_b, b_hh_b)]):
        w_ih = np.asarray(w_ih, np.float32)
        w_hh = np.asarray(w_hh, np.float32)
        bias = np.asarray(b_ih, np.float32) + np.asarray(b_hh, np.float32)
        for gs in range(4):
            pt = slot2pt[gs]
            rows = slice(pt * H, (pt + 1) * H)
            wih[d, gs, :E, :] = w_ih[rows, :].T
            wih[d, gs, E, :] = bias[rows]
            if gs < 2:
                wih[d, gs, E + 1, :] = PIN
            whh[d, gs, :, :] = w_hh[rows, :].T

    fc_w = np.asarray(fc_w, np.float32)
    fcw = np.stack([fc_w[:, :H].T.copy(), fc_w[:, H:].T.copy()])
    fcw16 = fcw.astype(np.float16)
    fcwl = (fcw - fcw16.astype(np.float32)).astype(np.float16)

    trans = np.asarray(trans, np.float32)
    fc_b = np.asarray(fc_b, np.float32)
    start_trans = np.asarray(start_trans, np.float32)

    C = (np.round(QS * (trans.T + fc_b[:, None] - DC)).astype(np.int64) * PACK
         + (T - 1 - np.arange(T))[None, :]).astype(np.int32)
    crep = np.broadcast_to(C, (128, T, T)).copy()

    initv = (np.round(QS * start_trans).astype(np.int64) * PACK).astype(np.int32)
    init_t = np.broadcast_to(initv, (128, T)).copy()

    rstr = np.full((128, T), RNULL, np.int32)
    rstm = np.zeros((128, T), np.int32)
    rstr_c0 = rstr.copy()
    rstm_c0 = rstm.copy()
    rstr_c0[0:64, :] = (np.round(QS * (start_trans + fc_b)).astype(np.int64)
                        * PACK).astype(np.int32)[None, :]
    rstm_c0[0:64, :] = RNULL

    sentence = np.asarray(sentence)
    base = {"wih": wih.astype(f16), "whh": whh.astype(f16),
            "fcw": fcw16, "fcwl": fcwl, "crep": crep, "init": init_t}
    in_maps = []
    for q in range(NC):
        cs = CL * q
        s_abs = cs + np.arange(NCOL) - WOFF
        ok = (s_abs >= 0) & (s_abs < S)
        toks = np.where(ok, sentence[:, np.clip(s_abs, 0, S - 1)], 0)
        x = ep[toks]                               # [B, NCOL, 128]
        x = x.transpose(2, 1, 0).copy()            # [128, NCOL, B]
        x[:, ~ok, :] = 0.0
        x[E + 1, ~ok, :] = 1.0
        x[E, ~ok, :] = 1.0
        m = dict(base)
        m["xT"] = np.ascontiguousarray(x.reshape(128, NCOL * B)).astype(f16)
        m["rstr"] = rstr_c0 if q == 0 else rstr
        m["rstm"] = rstm_c0 if q == 0 else rstm
        in_maps.append(m)
    return in_maps


def kernel(sentence, mask, embed, w_ih_f, w_hh_f, b_ih_f, b_hh_f,
           w_ih_b, w_hh_b, b_ih_b, b_hh_b, fc_w, fc_b,
           start_trans, end_trans, trans, _s_len=None, _profile=False,
           _dbg=False):
    sentence = np.asarray(sentence)
    assert sentence.shape == (B, S), "kernel is specialized to B=64, S=1024"
    assert _s_len in (None, S)
    nc = _get_nc(_dbg)
    in_maps = _host_inputs(sentence, embed, w_ih_f, w_hh_f, b_ih_f, b_hh_f,
                           w_ih_b, w_hh_b, b_ih_b, b_hh_b, fc_w, fc_b,
                           start_trans, end_trans, trans)
    res = run_bass_kernel_spmd(nc, in_maps, core_ids=list(range(NC)),
                               trace=_profile)

    hist_full = np.zeros((S, B, T), np.int32)
    for q in range(NC):
        hh = res.results[q]["hist_o"].reshape(2, 128, VW2, T)
        h0 = (T - 1) - (hh[0] & (PACK - 1))
        h1 = (T - 1) - (hh[1] & (PACK - 1))
        cs = CL * q
        hist_full[cs:cs + 32] = h0[0:64, 32:64, :].transpose(1, 0, 2)
        hist_full[cs + 32:cs + 64] = h1[0:64, 32:64, :].transpose(1, 0, 2)
        hist_full[cs + 64:cs + 96] = h0[64:128, 32:64, :].transpose(1, 0, 2)
        hist_full[cs + 96:cs + 128] = h1[64:128, 32:64, :].transpose(1, 0, 2)

    scf = res.results[NC - 1]["scf_o"][64:128, :].astype(np.float64)
    scf = scf / (QS * PACK) + np.asarray(end_trans, np.float64)[None, :]
    y = np.argmax(scf, axis=1)

    path = np.zeros((B, S), np.int64)
    path[:, S - 1] = y
    bi = np.arange(B)
    for s in range(S - 1, 0, -1):
        y = hist_full[s, bi, y]
        path[:, s - 1] = y
    out = path.astype(np.int32)
    if _profile or _dbg:
        return out, res
    return out
